# revision 32
# baseline (speedup 1.0000x reference)
"""Multi-head attention (12 heads, dh=64) + output projection on 8 TRN2 NeuronCores.

Strategy: pure data parallelism — B=8 batch elements, one per core. No collectives.
Each core computes the full attention layer for its batch element.

Precision: QK path (projection + scores) in fp16 (full PE rate, 11-bit mantissa;
bf16 q/k fails the 2e-2 rel-err gate because the peaked softmax amplifies logit
error ~0.028; fp16 lands at ~0.005). P/V/fc matmuls in bf16, f32 PSUM accumulation.

Per-core algorithm (N=1024 tokens, D=768, H=12, dh=64):
  1. qk projection, transposed layout: psqk[e,n] per head; V projection natural
     layout into vaug (bf16) with a ones-column per 65-wide head group so the
     P@V matmul also emits rowsum(P).
  2. pass1 per head: S[q,k] = qT.T @ kT (K=64); rowmax on DVE; PE-transpose of
     the [128,8] max matrix; DMA into row 64 of the augmented q tile.
  3. ST'[k,q] = kT_aug.T @ qT_aug with K=65 (row 64 = -1 / m[q] computes
     k.q - m[q] directly); exp on ACT (scale=8); OT_aug accumulated over kt
     with rowsum riding row 64.
  4. fc: out[n,d] = OT.T @ WfcT + b_fc.

Schedule (the performance core — PE must never idle or the HAM clock gate
re-throttles it from 2.4 GHz to 1.2 GHz):
  - Steady-state block h: [6 proj chunks h+2] [rs chain h-1] then kt 0..7 of
    {ST(h,kt); exp; one pass1 s_tile; PV(h,kt-2)}. PV runs TWO kt behind its
    exp so the PE never waits on ACT latency. psqk(h+2) evictions interleave
    on ACT/DVE at kts 0-2; mrow(h+1) fires at kt 4.
  - pass1(h) is split 5+3 across blocks h-2/h-1 so the m-row chain (last
    reduce -> transpose -> ACT copy -> sync-queue DMA) completes half a block
    before ST(h,0) consumes it.
  - psOT is released by a single 65-row DVE eviction (unnormalized OT + s);
    normalization is deferred one block: rs = exp(-ln s) computed on a
    [128,8] DRAM-bounce reshape (8-cycle ACT ops instead of 1024) for heads
    0-8, broadcast via zero-stride DRAM-source DMA, multiplied on the idle
    Pool engine. Heads 9-11 use a low-latency [1,1024] direct path on the
    sync DMA queue and DVE multiplies, because the fc consumes them
    immediately.
  - rs/s DMAs ride the GPSIMD SWDGE queue so they never head-of-line block
    the latency-critical m-row DMA on the sync queue.
  - fc: tile 0 prestarted inside head blocks 10/11 (chunks 0-3 banked to
    SBUF + bias, chunk 4 in block 11); tiles 1-7 pipeline chunk-5 one tile
    behind so nothing waits on the mul(11) chain. Redundant LDWEIGHTS of
    same-stationary matmul pairs are skipped (ins.ldweights=False).
"""

import os
import sys
from contextlib import ExitStack

import numpy as np

for _p in ("/opt/trn_rl_repo",):
    if _p not in sys.path and os.path.isdir(_p):
        sys.path.insert(0, _p)

import ml_dtypes  # noqa: E402

import concourse.bass as bass  # noqa: E402
import concourse.tile as tile  # noqa: E402
from concourse import mybir  # noqa: E402
from concourse.bass import ds, ts  # noqa: E402
from concourse.bass_utils import run_bass_kernel_spmd  # noqa: E402
from concourse.masks import make_identity  # noqa: E402

P = 128
NT = 1024   # tokens per core (batch element)
D = 768     # model dim
DC = D // P  # 6 contraction chunks
H = 12      # heads
DH = 64     # head dim
QT = NT // P  # 8 q tiles
KT = NT // P  # 8 k tiles
E3 = 3 * D  # 2304

F32 = mybir.dt.float32
F32R = mybir.dt.float32r
F16 = mybir.dt.float16
BF16 = mybir.dt.bfloat16

N_CORES = 8


def r(ap):
    """view an fp32 AP as float32r for full-speed PE matmul"""
    return ap.bitcast(F32R)


def no_ldw(inst):
    """Mark a matmul as reusing the stationary weights already loaded by the
    previous matmul (skips the redundant LDWEIGHTS; pairs sharing lhsT)."""
    inst.ins.ldweights = False
    return inst



def _split_sync_waits(nc, max_waits=1):
    """Walrus codegen allows only a limited number of semaphore waits per
    instruction (one for several instruction structs). Move extra waits onto
    same-engine NoOps inserted immediately before the offending instruction."""
    from concourse import mybir as mb
    for f in nc.m.functions:
        for b in f.blocks:
            out = []
            changed = False
            for inst in b.instructions:
                si = inst.sync_info
                waits = list(si.on_wait) if (si is not None and si.on_wait) else []
                eng = getattr(inst, "engine", None)
                if (type(inst).__name__ == "InstISA"
                        and getattr(inst, "op_name", None) == "EVENT_SEMAPHORE_RANGE_CLEAR"):
                    # walrus here rejects this opcode; emit per-sem resets instead
                    lo, hi = inst.instr[13], inst.instr[14]
                    for sid in range(lo, hi + 1):
                        out.append(mb.InstEventSemaphore(
                            name=nc.get_next_instruction_name(),
                            sync_info=mb.SyncInfo(on_wait=[], on_update=[
                                mb.SyncUpdate(sync_type="semaphore", id=sid,
                                              ant_name=f"semclr_{sid}",
                                              update_mode="sem-wr-imm",
                                              update_value=0, update_reg=None)]),
                            engine=eng,
                        ))
                    changed = True
                    continue
                lim = max_waits
                if len(waits) > lim and eng is not None:
                    for w in waits[:-lim]:
                        nop = mb.InstEventSemaphore(
                            name=nc.get_next_instruction_name(),
                            sync_info=mb.SyncInfo(on_wait=[w], on_update=[]),
                            engine=eng,
                        )
                        out.append(nop)
                    inst.sync_info = mb.SyncInfo(
                        on_wait=waits[-lim:],
                        on_update=list(si.on_update) if si.on_update else [],
                    )
                    changed = True
                out.append(inst)
            if changed:
                b.instructions = out


def build_graph():
    nc = bass.Bass()
    imgT = nc.declare_dram_parameter("imgT", [D, NT], F16, isOutput=False)
    WqkvT = nc.declare_dram_parameter("WqkvT", [D, E3], F16, isOutput=False)
    WfcT = nc.declare_dram_parameter("WfcT", [D, D], BF16, isOutput=False)
    b_fc = nc.declare_dram_parameter("b_fc", [D], F32, isOutput=False)
    out = nc.declare_dram_parameter("out", [NT, D], F32, isOutput=True)

    with tile.TileContext(nc) as tc, ExitStack() as ctx:
        const = ctx.enter_context(tc.tile_pool(name="const", bufs=1))
        ptp = ctx.enter_context(tc.tile_pool(name="ptp", bufs=4))
        small = ctx.enter_context(tc.tile_pool(name="small", bufs=2))
        rawp = ctx.enter_context(tc.tile_pool(name="rawp", bufs=3))
        rsp = ctx.enter_context(tc.tile_pool(name="rsp", bufs=3))
        outp = ctx.enter_context(tc.tile_pool(name="outp", bufs=3))
        # PSUM: two pools of 2 x [128,1024] slots = 8 banks total.
        # bigp: high-churn stream tiles (S, ST', mrow transpose), accp:
        # longer-lived accumulators (psqk proj, psOT).
        bigp = ctx.enter_context(tc.tile_pool(name="bigp", bufs=2, space="PSUM"))
        accp = ctx.enter_context(tc.tile_pool(name="accp", bufs=2, space="PSUM"))

        # ---- input loads: interleave img/wq chunks so the first projection
        # can start as soon as chunk 0 of each has landed; wf/bias last ----
        img_sb = []
        wq_sb = []
        wf_sb = []
        for c in range(DC):
            ti = const.tile([P, NT], F16, tag=f"img{c}", name=f"img{c}")
            nc.sync.dma_start(out=ti[:, :], in_=imgT[ds(c * P, P), :])
            img_sb.append(ti)
            tw = const.tile([P, E3], F16, tag=f"wq{c}", name=f"wq{c}")
            nc.sync.dma_start(out=tw[:, :], in_=WqkvT[ds(c * P, P), :])
            wq_sb.append(tw)
        for c in range(DC):
            t = const.tile([P, D], BF16, tag=f"wf{c}")
            nc.sync.dma_start(out=t[:, :], in_=WfcT[ds(c * P, P), :])
            wf_sb.append(t)

        bias_sb = const.tile([P, D], F32, tag="bias")
        b_ap = b_fc[:]
        b_bcast = bass.AP(tensor=b_ap.tensor, offset=b_ap.offset,
                          ap=[[0, P]] + list(b_ap.ap))
        nc.sync.dma_start(out=bias_sb[:, :], in_=b_bcast)

        # ---- HAM pre-warming ----
        # The PE clock-gate (HAM) releases 2.4 GHz only after ~3.4us of
        # sustained activity; engine init (~7.5us) + the input-DMA window
        # otherwise leave the PE cold+idle until ~12us, so the whole warmup
        # runs at 1.2 GHz. Burn the idle DMA-wait on dummy matmuls over a
        # zeroed tile (memset emitted first so nothing delays them): they
        # finish right as the first real chunk lands and the V projection
        # starts at full clock.
        warm_src = const.tile([P, 512], F16, tag="warm")
        nc.vector.memset(warm_src[:, :], 0.0)
        for i in range(16):
            psW = bigp.tile([P, 512], F32, tag="big", name=f"warm{i}")
            nc.tensor.matmul(psW[:, :], warm_src[:, 0:128], warm_src[:, :],
                             start=True, stop=True)

        ident = const.tile([P, P], F32, tag="ident")
        make_identity(nc, ident[:, :])

        # V with ones column per head: [k-part, kt, h*65 + c], col 64 of each
        # group = 1. Only the ones columns need initialization (the rest is
        # overwritten by the V-projection eviction) -> strided DVE memset.
        vaug = const.tile([P, KT, H * 65], BF16, tag="vaug")
        nc.vector.memset(
            vaug[:, :, :].rearrange("p t (h c) -> p t h c", c=65)[:, :, :, 64:65],
            1.0)

        # merged attention output, transposed, one tile per fc contraction
        # chunk (separate tiles so fc chunk c only depends on the two head
        # muls that write it, not on every head via coarse tile tracking)
        otc = [const.tile([P, NT], BF16, tag=f"otc{c}", name=f"otc{c}")
               for c in range(DC)]

        # persistent triple-buffered q/k tiles (parity h%3; three-deep so the
        # mid-block eviction of head h+2 never overlaps head h's readers).
        # ka row 64 = -1 (set once); qa row 64 = rowmax, DMA'd in by mrow().
        qa_bufs = [const.tile([65, NT], F16, tag=f"qab{i}", name=f"qab{i}")
                   for i in range(3)]
        ka_bufs = [const.tile([65, NT], F16, tag=f"kab{i}", name=f"kab{i}")
                   for i in range(3)]
        for i in range(3):
            nc.vector.memset(ka_bufs[i][64:65, :], -1.0)

        # ---- V projection (natural layout) ----
        for t in range(QT):
            psv = bigp.tile([P, D], F32, tag="big", name=f"psv{t}")
            for c in range(DC):
                lt = img_sb[c][:, ts(t, P)]
                wv = wq_sb[c][:, :].rearrange("p (h x) -> p h x", h=H)
                nc.tensor.matmul(psv[:, 0:512].rearrange("p (h x) -> p h x", h=8),
                                 lt, wv[:, 0:8, 128:192],
                                 start=(c == 0), stop=(c == DC - 1))
                no_ldw(nc.tensor.matmul(
                    psv[:, 512:768].rearrange("p (h x) -> p h x", h=4),
                    lt, wv[:, 8:12, 128:192],
                    start=(c == 0), stop=(c == DC - 1)))
            nc.scalar.copy(
                vaug[:, t, :].rearrange("p (h x) -> p h x", h=H)[:, :, 0:64],
                psv[:, :].rearrange("p (h x) -> p h x", h=H))

        def proj_alloc(h):
            return accp.tile([P, NT], F32, tag="acc", name=f"psqk{h}")

        def proj_chunk(psqk, h, c):
            lt = wq_sb[c][:, ds(h * 192, P)]
            for nb in range(2):
                i = nc.tensor.matmul(psqk[:, ts(nb, 512)], lt,
                                     img_sb[c][:, ts(nb, 512)],
                                     start=(c == 0), stop=(c == DC - 1))
                if nb:
                    no_ldw(i)

        def proj_evict(psqk, h):
            """fp32 PSUM -> fp16 qa/ka; split across ACT and DVE."""
            qa = qa_bufs[h % 3]
            ka = ka_bufs[h % 3]
            nc.scalar.copy(qa[0:64, :], psqk[0:64, :])
            nc.vector.tensor_copy(ka[0:64, :], psqk[64:128, :])
            return qa, ka

        def evict_qa_half(psqk, h, half):
            """half-width ACT eviction, interleaved between exp ops so the
            softmax stream never stalls more than one exp slot."""
            qa = qa_bufs[h % 3]
            nc.scalar.copy(qa[0:64, ts(half, 512)], psqk[0:64, ts(half, 512)])

        def evict_ka(psqk, h):
            ka = ka_bufs[h % 3]
            nc.vector.tensor_copy(ka[0:64, :], psqk[64:128, :])
            return qa_bufs[h % 3], ka

        def proj_head(h):
            psqk = proj_alloc(h)
            for c in range(DC):
                proj_chunk(psqk, h, c)
            return proj_evict(psqk, h)

        def s_tile(t, qa, ka, mcat, nm=""):
            """S[q,k] for one q-tile; rowmax on DVE. (tensor_tensor_reduce on
            the two halves would halve the cycles but the ISA allows only one
            PSUM input per DVE instruction.)"""
            psS = bigp.tile([P, NT], F32, tag="big", name=f"psS{nm}_{t}")
            lt = qa[0:64, ts(t, P)]
            for nb in range(2):
                i = nc.tensor.matmul(psS[:, ts(nb, 512)], lt,
                                     ka[0:64, ts(nb, 512)],
                                     start=True, stop=True)
                if nb:
                    no_ldw(i)
            nc.vector.reduce_max(out=mcat[:, ds(t, 1)], in_=psS[:, :],
                                 axis=mybir.AxisListType.X)

        def mrow(h, qa, mcat):
            """m [128,8] -> transposed row [1,1024] into qa row 64"""
            psT = bigp.tile([8, P], F32, tag="big", name=f"psT{h}")
            nc.tensor.transpose(psT[:, :], mcat[:, :], ident[:, :])
            m_sb = small.tile([8, P], F16, tag="mrow")
            nc.scalar.copy(m_sb[:, :], psT[:, :])
            nc.sync.dma_start(
                out=qa[ds(64, 1), :].rearrange("o (t x) -> o t x", t=QT),
                in_=m_sb[:, :])

        raws = {}
        rs64s = {}

        def rs_chain(h, psOT, dma_eng=None):
            """Free psOT fast with one 65-row DVE eviction (OT + rowsum s).
            rs = 1/s via exp(-ln s), but computed on a [128, 8] RESHAPE of the
            s row (DMA bounce through DRAM) so the two ACT ops cost 8 cycles
            of free dim instead of 1024 -- the ACT engine stays exp-bound.
            Then flatten + partition-broadcast back (DRAM source allows the
            zero-stride partition dim). All DMAs ride the GPSIMD SWDGE queue
            (sync queue for the last head, which feeds the fc with the sync
            queue idle) so they never block the m-row DMA."""
            raw = rawp.tile([65, NT], BF16, tag="raw", name=f"raw{h}")
            nc.vector.tensor_copy(raw[:, 0:512], psOT[:, 0:512])
            nc.scalar.copy(raw[:, 512:1024], psOT[:, 512:1024])
            raws[h] = raw
            eng = dma_eng if dma_eng is not None else nc.gpsimd
            if h >= 9:
                # low-latency direct path: Ln straight off the PSUM row (no
                # raw dependency), one bounce + one broadcast. These heads
                # gate the fc prestart/tail.
                lns1 = small.tile([1, NT], F32, tag="lns1")
                nc.scalar.activation(lns1[:, :], psOT[ds(64, 1), :],
                                     mybir.ActivationFunctionType.Ln,
                                     bias=0.0, scale=1.0)
                rs1 = small.tile([1, NT], F16, tag="rs1")
                nc.scalar.activation(rs1[:, :], lns1[:, :],
                                     mybir.ActivationFunctionType.Exp,
                                     bias=0.0, scale=-1.0)
                rd = nc.dram_tensor(f"rs_dram{h}", [NT], F16, kind="Internal")
                eng.dma_start(out=rd[:], in_=rs1[:, :])
                rs64d = rsp.tile([DH, NT], F16, tag="rs64", name=f"rs64_{h}")
                dd = rd[:]
                bcd = bass.AP(tensor=dd.tensor, offset=dd.offset,
                              ap=[[0, DH]] + list(dd.ap))
                eng.dma_start(out=rs64d[:, :], in_=bcd)
                rs64s[h] = rs64d
                return
            s_dram = nc.dram_tensor(f"s_dram{h}", [NT], BF16, kind="Internal")
            eng.dma_start(out=s_dram[:], in_=raw[ds(64, 1), :])
            s128 = small.tile([P, QT], BF16, tag="s128")
            eng.dma_start(out=s128[:, :], in_=s_dram[:])
            lns = small.tile([P, QT], F32, tag="lns")
            nc.scalar.activation(lns[:, :], s128[:, :],
                                 mybir.ActivationFunctionType.Ln,
                                 bias=0.0, scale=1.0)
            rs128 = small.tile([P, QT], F16, tag="rs128")
            nc.scalar.activation(rs128[:, :], lns[:, :],
                                 mybir.ActivationFunctionType.Exp,
                                 bias=0.0, scale=-1.0)
            rs_dram = nc.dram_tensor(f"rs_dram{h}", [NT], F16, kind="Internal")
            eng.dma_start(out=rs_dram[:], in_=rs128[:, :])
            rs64 = rsp.tile([DH, NT], F16, tag="rs64", name=f"rs64_{h}")
            d_ap = rs_dram[:]
            r_bcast = bass.AP(tensor=d_ap.tensor, offset=d_ap.offset,
                              ap=[[0, DH]] + list(d_ap.ap))
            eng.dma_start(out=rs64[:, :], in_=r_bcast)
            rs64s[h] = rs64

        def mul_head(h):
            """deferred normalization multiply: heads 0-8 on the idle Pool
            engine (one-block latency budget), 9-11 on DVE (the fc prestart
            and tail consume them with little slack)."""
            dst = otc[h // 2][ds((h % 2) * 64, DH), :]
            if h <= 8:
                nc.gpsimd.tensor_mul(dst, raws[h][0:64, :], rs64s[h][:, :])
            else:
                nc.vector.tensor_mul(dst, raws[h][0:64, :], rs64s[h][:, :])
            del raws[h], rs64s[h]

        def pv_mms(h, kt, pt_t, psOT):
            for nb in range(2):
                i = nc.tensor.matmul(psOT[:, ts(nb, 512)],
                                     vaug[:, kt, ds(h * 65, 65)],
                                     pt_t[:, ts(nb, 512)],
                                     start=(kt == 0), stop=(kt == KT - 1))
                if nb:
                    no_ldw(i)

        # ---- warmup: heads 0 and 1 projected; pass1(0) complete (+mrow),
        # pass1(1) tiles 0-4 done (remainder lands in block 0) ----
        qa_ka = {}
        mcats = {}
        s_q = {}
        qa_ka[0] = proj_head(0)
        mcats[0] = small.tile([P, QT], F32, tag="mcat", name="mcat0")
        for t in range(QT):
            s_tile(t, qa_ka[0][0], qa_ka[0][1], mcats[0], nm="w")
        mrow(0, qa_ka[0][0], mcats[0])
        qa_ka[1] = proj_head(1)
        mcats[1] = small.tile([P, QT], F32, tag="mcat", name="mcat1")
        s_q[1] = list(range(QT))
        for _ in range(5):
            t = s_q[1].pop(0)
            s_tile(t, qa_ka[1][0], qa_ka[1][1], mcats[1], nm="1")

        # fc-prestart state (tile 0 of the fc is computed early to give the
        # last two head blocks PE filler): fcpart0 = chunks 0-3 + bias.
        fcpart0 = const.tile([P, D], F32, tag="fcpart0")
        fc_state = {}
        pending_rs = {}

        def fc_chunk(psF, t, c, start, stop):
            nc.tensor.matmul(psF[:, 0:512], otc[c][:, ts(t, P)],
                             wf_sb[c][:, 0:512], start=start, stop=stop)
            no_ldw(nc.tensor.matmul(psF[:, 512:768], otc[c][:, ts(t, P)],
                                    wf_sb[c][:, 512:768],
                                    start=start, stop=stop))

        # ---- steady-state head blocks ----
        # Block h (PE program order):
        #   pre: [6 proj chunks h+2] [rs chain h-1 (ACT/DVE/DMA only)]
        #   kt 0..2: ST | s_tile(h+1) remainder | PV        (+ psqk(h+2)
        #     evictions on ACT/DVE at kts 1-2, ka first)
        #   kt 3..7: ST | s_tile(h+2) tiles 0..4 | PV; mrow(h+1) at kt 4.
        #   tail: deferred norm muls.
        # pass1(h) is split 5+3 across blocks h-2/h-1 so the m-row chain
        # (last reduce -> PE transpose -> ACT copy -> DMA) completes half a
        # block before ST(h,0) consumes qa(h) row 64.
        for h in range(H):
            qa, ka = qa_ka[h]
            nxt = qa_ka.get(h + 1)
            prj = h + 2 if h + 2 < H else None
            # accp allocation order per block must stay strictly alternating:
            # normal blocks [psqk(h+2), psOT(h)]; h=10 [psOT, psF0];
            # h=11 [psOT, psF0b].
            psqk2 = proj_alloc(prj) if prj is not None else None
            if psqk2 is None:
                psOT = accp.tile([65, NT], F32, tag="acc", name=f"psOT{h}")
                psF0 = accp.tile([P, D], F32, tag="acc", name=f"psF0_{h}")
                fc_state[h] = psF0
            # --- pre-region ---
            if psqk2 is not None:
                for c in range(DC):
                    proj_chunk(psqk2, prj, c)
            elif h == 10:
                psF0 = fc_state[10]
                for c in range(4):
                    fc_chunk(psF0, 0, c, start=(c == 0), stop=(c == 3))
            elif h == 11:
                psF0b = fc_state[11]
                fc_chunk(psF0b, 0, 4, start=True, stop=False)
            if (h - 1) in pending_rs:
                rs_chain(h - 1, pending_rs.pop(h - 1))
            if psqk2 is not None:
                psOT = accp.tile([65, NT], F32, tag="acc", name=f"psOT{h}")
            if prj is not None:
                mcats[prj] = small.tile([P, QT], F32, tag="mcat",
                                        name=f"mcat{prj}")
                s_q[prj] = list(range(QT))
            # --- softmax/PV pipeline ---
            pts = {}
            for kt in range(KT):
                psST = bigp.tile([P, NT], F32, tag="big", name=f"psST{h}_{kt}")
                lt = ka[:, ts(kt, P)]
                for nb in range(2):
                    i = nc.tensor.matmul(psST[:, ts(nb, 512)], lt,
                                         qa[:, ts(nb, 512)],
                                         start=True, stop=True)
                    if nb:
                        no_ldw(i)
                pt_t = ptp.tile([P, NT], BF16, tag="pt")
                nc.scalar.activation(pt_t[:, :], psST[:, :],
                                     mybir.ActivationFunctionType.Exp,
                                     bias=0.0, scale=8.0)
                pts[kt] = pt_t
                # fillers: kts 0-2 finish pass1(h+1); kts 3-7 run the first
                # five tiles of pass1(h+2)
                if kt < 3:
                    if nxt is not None and s_q.get(h + 1):
                        t = s_q[h + 1].pop(0)
                        s_tile(t, nxt[0], nxt[1], mcats[h + 1], nm=str(h + 1))
                elif prj is not None and s_q.get(prj):
                    t = s_q[prj].pop(0)
                    s_tile(t, qa_ka[prj][0], qa_ka[prj][1], mcats[prj],
                           nm=str(prj))
                if psqk2 is not None:
                    if kt == 0:
                        qa_ka[prj] = evict_ka(psqk2, prj)
                    elif kt == 1:
                        evict_qa_half(psqk2, prj, 0)
                    elif kt == 2:
                        evict_qa_half(psqk2, prj, 1)
                if kt == 4 and nxt is not None:
                    mrow(h + 1, nxt[0], mcats[h + 1])
                if kt > 1:
                    pv_mms(h, kt - 2, pts[kt - 2], psOT)
                    del pts[kt - 2]
            pv_mms(h, KT - 2, pts[KT - 2], psOT)
            pv_mms(h, KT - 1, pts[KT - 1], psOT)
            # --- tail ---
            if h == H - 1:
                rs_chain(h, psOT, dma_eng=nc.sync)
            else:
                pending_rs[h] = psOT
            if h == 10:
                # bank partial fc tile 0 (chunks 0-3 + bias) into SBUF so the
                # accp slot frees this block (avoids a psOT(11) ring stall)
                nc.vector.tensor_add(fcpart0[:, :], fc_state[10][:, :],
                                     bias_sb[:, :])
            if h > 0:
                mul_head(h - 1)

        # ---- fc + bias, two-phase ----
        # Phase 1: chunks 0-3 (+bias) for tiles 1-7, banked to SBUF — no
        # dependency on the late heads, so ~12us of PE work hides the
        # mul(11) normalization chain completely. mul(11) is emitted AFTER
        # the phase-1 adds so it never head-of-line blocks the DVE queue.
        # Phase 2: chunks 4-5 + combine with the banked partial. Tile 0's
        # phase 1 ran as filler inside head blocks 10/11.
        fcparts = {0: fcpart0}

        def fc_phase1(t):
            # chunks 0-4: heads 0-9, all normalized well before the fc
            psF = bigp.tile([P, D], F32, tag="big", name=f"psF{t}")
            for c in range(5):
                fc_chunk(psF, t, c, start=(c == 0), stop=(c == 4))
            fp = const.tile([P, D], F32, tag=f"fcp{t}", name=f"fcp{t}")
            nc.vector.tensor_add(fp[:, :], psF[:, :], bias_sb[:, :])
            fcparts[t] = fp

        def fc_phase2(t):
            if t == 0:
                psF = fc_state[11]  # chunk 4 accumulated in block 11's pre
                fc_chunk(psF, t, 5, start=False, stop=True)
            else:
                psF = bigp.tile([P, D], F32, tag="big", name=f"psF2_{t}")
                fc_chunk(psF, t, 5, start=True, stop=True)
            o_t = outp.tile([P, D], F32, tag="o", name=f"o_t{t}")
            nc.vector.tensor_add(o_t[:, :], psF[:, :], fcparts[t][:, :])
            nc.sync.dma_start(out=out[ts(t, P), :], in_=o_t[:, :])

        for t in range(1, QT):
            fc_phase1(t)
        mul_head(H - 1)
        for t in range(QT):
            fc_phase2(t)

    _split_sync_waits(nc)
    return nc


_NC_CACHE = {}


def _get_graph():
    if "nc" not in _NC_CACHE:
        _NC_CACHE["nc"] = build_graph()
    return _NC_CACHE["nc"]


_EXEC_CACHE = {}


def _install_compile_memo():
    import hashlib
    import shutil
    from concourse import bass_utils as bu
    from concourse import bass2jax
    if getattr(bu.compile_bir_kernel, "_memo", False):
        return
    orig = bu.compile_bir_kernel

    def memo_compile(bir_json, tmpdir, neff_name="file.neff"):
        key = hashlib.sha256(bir_json).hexdigest()
        os.makedirs("/tmp/neff_cache", exist_ok=True)
        persist = f"/tmp/neff_cache/{key}.neff"
        if os.path.exists(persist):
            return persist
        r = orig(bir_json, tmpdir, neff_name)
        shutil.copyfile(r, persist)
        return persist
    memo_compile._memo = True
    bu.compile_bir_kernel = memo_compile
    bass2jax.compile_bir_kernel = memo_compile


def _get_executor():
    _install_compile_memo()
    """Build (once) a jitted shard_map executor over 8 cores, non-donating so
    it can be re-invoked for benchmarking. Mirrors bass2jax.run_bass_via_pjrt."""
    if "exec" in _EXEC_CACHE:
        return _EXEC_CACHE["exec"]
    import jax
    import jax.numpy as jnp
    from jax.sharding import Mesh, PartitionSpec
    from jax.experimental.shard_map import shard_map
    from concourse import mybir as mb
    from concourse import bass2jax

    bass2jax.install_neuronx_cc_hook()
    nc = _get_graph()
    partition_name = (nc.partition_id_tensor.name
                      if nc.partition_id_tensor else None)
    in_names, out_names, out_avals, zero_outs = [], [], [], []
    for alloc in nc.m.functions[0].allocations:
        if not isinstance(alloc, mb.MemoryLocationSet):
            continue
        name = alloc.memorylocations[0].name
        if alloc.kind == "ExternalInput":
            if name != partition_name:
                in_names.append(name)
        elif alloc.kind == "ExternalOutput":
            shape = tuple(alloc.tensor_shape)
            dtype = mb.dt.np(alloc.dtype)
            out_names.append(name)
            out_avals.append(jax.core.ShapedArray(shape, dtype))
            zero_outs.append(np.zeros(shape, dtype))
    n_params = len(in_names)
    all_in_names = list(in_names) + list(out_names)
    if partition_name is not None:
        all_in_names.append(partition_name)

    def _body(*args):
        operands = list(args)
        if partition_name is not None:
            operands.append(bass2jax.partition_id_tensor())
        outs = bass2jax._bass_exec_p.bind(
            *operands,
            out_avals=tuple(out_avals),
            in_names=tuple(all_in_names),
            out_names=tuple(out_names),
            lowering_input_output_aliases=(),
            sim_require_finite=True,
            sim_require_nnan=True,
            nc=nc,
        )
        return tuple(outs)

    devices = jax.devices()[:N_CORES]
    mesh = Mesh(np.asarray(devices), ("core",))
    n_outs = len(out_names)
    in_specs = (PartitionSpec("core"),) * (n_params + n_outs)
    out_specs = (PartitionSpec("core"),) * n_outs
    sharded = jax.jit(shard_map(_body, mesh=mesh, in_specs=in_specs,
                                out_specs=out_specs, check_rep=False))
    ex = dict(fn=sharded, in_names=in_names, out_names=out_names,
              out_avals=out_avals, zero_outs=zero_outs, n_params=n_params)
    _EXEC_CACHE["exec"] = ex
    return ex


def _prep_inputs(img, W_qkv, W_fc, b_fc):
    img = np.asarray(img, dtype=np.float32)
    W_qkv = np.asarray(W_qkv, dtype=np.float32)
    W_fc = np.asarray(W_fc, dtype=np.float32)
    b_fc = np.asarray(b_fc, dtype=np.float32)
    imgT = np.ascontiguousarray(img.transpose(0, 2, 1)).astype(np.float16)
    WqkvT = np.ascontiguousarray(W_qkv.T).astype(np.float16)
    WfcT = np.ascontiguousarray(W_fc.T).astype(ml_dtypes.bfloat16)
    return [{"imgT": imgT[i], "WqkvT": WqkvT, "WfcT": WfcT, "b_fc": b_fc}
            for i in range(N_CORES)]


def _run_cached(in_maps):
    ex = _get_executor()
    n_cores = N_CORES
    per_core = [[np.asarray(m[name]) for name in ex["in_names"]]
                for m in in_maps]
    concat_in = [np.concatenate([per_core[c][i] for c in range(n_cores)], axis=0)
                 for i in range(ex["n_params"])]
    concat_zeros = [np.zeros((n_cores * z.shape[0], *z.shape[1:]), z.dtype)
                    for z in ex["zero_outs"]]
    out_arrs = ex["fn"](*concat_in, *concat_zeros)
    outs = [
        {name: np.asarray(out_arrs[i]).reshape(n_cores, *ex["out_avals"][i].shape)[c]
         for i, name in enumerate(ex["out_names"])}
        for c in range(n_cores)
    ]
    return outs


def bench(n_iters=20):
    """Wall-clock benchmark of the compiled executable (inputs device-resident
    once; n_iters sequential executes, block at the end)."""
    import time
    import jax
    inputs_np = None
    try:
        z = np.load("/root/problem/_expected.npz")
        inputs_np = {k: z[k] for k in ("img", "W_qkv", "W_fc", "b_fc")}
    except Exception:
        rng = np.random.default_rng(0)
        inputs_np = {
            "img": rng.standard_normal((8, 1024, 768), dtype=np.float32),
            "W_qkv": rng.standard_normal((E3, D), dtype=np.float32),
            "W_fc": rng.standard_normal((D, D), dtype=np.float32),
            "b_fc": rng.standard_normal((D,), dtype=np.float32),
        }
    in_maps = _prep_inputs(**inputs_np)
    ex = _get_executor()
    per_core = [[np.asarray(m[name]) for name in ex["in_names"]] for m in in_maps]
    concat_in = [np.concatenate([per_core[c][i] for c in range(N_CORES)], axis=0)
                 for i in range(ex["n_params"])]
    concat_zeros = [np.zeros((N_CORES * z.shape[0], *z.shape[1:]), z.dtype)
                    for z in ex["zero_outs"]]
    # warmup + compile
    o = ex["fn"](*concat_in, *concat_zeros)
    jax.block_until_ready(o)
    # sequential, block each call
    ts = []
    for _ in range(n_iters):
        t0 = time.perf_counter()
        o = ex["fn"](*concat_in, *concat_zeros)
        jax.block_until_ready(o)
        ts.append(time.perf_counter() - t0)
    # pipelined: fire all, block once
    t0 = time.perf_counter()
    os_ = [ex["fn"](*concat_in, *concat_zeros) for _ in range(n_iters)]
    jax.block_until_ready(os_)
    piped = (time.perf_counter() - t0) / n_iters
    return dict(min_s=min(ts), mean_s=sum(ts) / len(ts), piped_s=piped)


def _run(img, W_qkv, W_fc, b_fc, trace=False, tmpdir=None):
    in_maps = _prep_inputs(img, W_qkv, W_fc, b_fc)
    results = _run_cached(in_maps)
    outs = np.stack([np.asarray(results[i]["out"], dtype=np.float32)
                     for i in range(N_CORES)])
    return outs, None


def kernel(img, W_qkv, W_fc, b_fc):
    outs, _ = _run(img, W_qkv, W_fc, b_fc)
    return outs


def bench_chain(n=9, reps=5):
    """Real-HW per-iteration time: jit a chain of n dependent kernel
    executions (out buffer of call i feeds call i+1). Slope = (t_n - t_1)/(n-1).
    Walrus compiles are memoized by BIR hash so the chain compiles once."""
    import time
    import hashlib
    import jax
    from jax.sharding import Mesh, PartitionSpec
    from jax.experimental.shard_map import shard_map
    from concourse import mybir as mb
    from concourse import bass2jax
    from concourse import bass_utils as bu

    if not hasattr(bu.compile_bir_kernel, "_memo"):
        orig = bu.compile_bir_kernel

        def memo_compile(bir_json, tmpdir, neff_name="file.neff"):
            import shutil
            key = hashlib.sha256(bir_json).hexdigest()
            cache = memo_compile._cache
            if key in cache:
                return cache[key]
            r = orig(bir_json, tmpdir, neff_name)
            os.makedirs("/tmp/neff_cache", exist_ok=True)
            persist = f"/tmp/neff_cache/{key}.neff"
            shutil.copyfile(r, persist)
            cache[key] = persist
            return persist
        memo_compile._cache = {}
        memo_compile._memo = True
        bu.compile_bir_kernel = memo_compile
        bass2jax.compile_bir_kernel = memo_compile

    bass2jax.install_neuronx_cc_hook()
    nc = _get_graph()
    partition_name = (nc.partition_id_tensor.name
                      if nc.partition_id_tensor else None)
    in_names, out_names, out_avals = [], [], []
    for alloc in nc.m.functions[0].allocations:
        if not isinstance(alloc, mb.MemoryLocationSet):
            continue
        name = alloc.memorylocations[0].name
        if alloc.kind == "ExternalInput":
            if name != partition_name:
                in_names.append(name)
        elif alloc.kind == "ExternalOutput":
            out_names.append(name)
            out_avals.append(jax.core.ShapedArray(
                tuple(alloc.tensor_shape), mb.dt.np(alloc.dtype)))
    n_params = len(in_names)
    all_in = list(in_names) + list(out_names)
    if partition_name is not None:
        all_in.append(partition_name)

    def make_body(n_iter):
        def _body(*args):
            ins = list(args[:n_params])
            outb = list(args[n_params:])
            outs = None
            for _ in range(n_iter):
                operands = ins + outb
                if partition_name is not None:
                    operands.append(bass2jax.partition_id_tensor())
                outs = bass2jax._bass_exec_p.bind(
                    *operands,
                    out_avals=tuple(out_avals),
                    in_names=tuple(all_in),
                    out_names=tuple(out_names),
                    lowering_input_output_aliases=(),
                    sim_require_finite=False,
                    sim_require_nnan=False,
                    nc=nc,
                )
            return tuple(outs)
        return _body

    z = np.load("/root/problem/_expected.npz")
    in_maps = _prep_inputs(z["img"], z["W_qkv"], z["W_fc"], z["b_fc"])
    per_core = [[np.asarray(m[k]) for k in in_names] for m in in_maps]
    concat_in = [np.concatenate([per_core[c][i] for c in range(N_CORES)], axis=0)
                 for i in range(n_params)]
    concat_zeros = [np.zeros((N_CORES * a.shape[0], *a.shape[1:]), a.dtype)
                    for a in out_avals]
    devices = jax.devices()[:N_CORES]
    mesh = Mesh(np.asarray(devices), ("core",))
    res = {}
    for n_iter in (1, n):
        body = make_body(n_iter)
        fn = jax.jit(shard_map(body, mesh=mesh,
                               in_specs=(PartitionSpec("core"),) * (n_params + len(out_names)),
                               out_specs=(PartitionSpec("core"),) * len(out_names),
                               check_rep=False))
        o = fn(*concat_in, *concat_zeros)
        jax.block_until_ready(o)  # warm
        ts = []
        for _ in range(reps):
            t0 = time.perf_counter()
            o = fn(*concat_in, *concat_zeros)
            jax.block_until_ready(o)
            ts.append(time.perf_counter() - t0)
        res[n_iter] = min(ts)
        print(f"chain n={n_iter}: min {min(ts)*1e3:.2f} ms over {reps} reps")
    per_iter = (res[n] - res[1]) / (n - 1)
    print(f"per-iteration (HW exec) ~= {per_iter*1e6:.1f} us")
    return per_iter


def bench_resident(m1=10, m2=40):
    """Per-call cost with device-resident inputs and a single executable:
    slope between m1 and m2 sequential async dispatches."""
    import time
    import jax
    from jax.sharding import Mesh, PartitionSpec, NamedSharding
    ex = _get_executor()
    z = np.load("/root/problem/_expected.npz")
    in_maps = _prep_inputs(z["img"], z["W_qkv"], z["W_fc"], z["b_fc"])
    per_core = [[np.asarray(m[k]) for k in ex["in_names"]] for m in in_maps]
    concat_in = [np.concatenate([per_core[c][i] for c in range(N_CORES)], axis=0)
                 for i in range(ex["n_params"])]
    concat_zeros = [np.zeros((N_CORES * z_.shape[0], *z_.shape[1:]), z_.dtype)
                    for z_ in ex["zero_outs"]]
    devices = jax.devices()[:N_CORES]
    mesh = Mesh(np.asarray(devices), ("core",))
    sh = NamedSharding(mesh, PartitionSpec("core"))
    dev_in = [jax.device_put(a, sh) for a in concat_in]
    dev_zero = [jax.device_put(a, sh) for a in concat_zeros]
    jax.block_until_ready(dev_in + dev_zero)
    fn = ex["fn"]
    o = fn(*dev_in, *dev_zero)
    jax.block_until_ready(o)
    res = {}
    for m in (m1, m2):
        best = None
        for _ in range(3):
            t0 = time.perf_counter()
            outs = [fn(*dev_in, *dev_zero) for _ in range(m)]
            jax.block_until_ready(outs)
            dt = time.perf_counter() - t0
            best = dt if best is None else min(best, dt)
        res[m] = best
        print(f"m={m}: {best*1e3:.2f} ms total, {best/m*1e3:.3f} ms/call")
    slope = (res[m2] - res[m1]) / (m2 - m1)
    print(f"slope (per-call device cost) ~= {slope*1e6:.1f} us")
    return slope



# revision 34
# speedup vs baseline: 1.0090x; 1.0090x over previous
"""Multi-head attention (12 heads, dh=64) + output projection on 8 TRN2 NeuronCores.

Strategy: pure data parallelism — B=8 batch elements, one per core. No collectives.
Each core computes the full attention layer for its batch element.

Precision: QK path (projection + scores) in fp16 (full PE rate, 11-bit mantissa;
bf16 q/k fails the 2e-2 rel-err gate because the peaked softmax amplifies logit
error ~0.028; fp16 lands at ~0.005). P/V/fc matmuls in bf16, f32 PSUM accumulation.

Per-core algorithm (N=1024 tokens, D=768, H=12, dh=64):
  1. qk projection, transposed layout: psqk[e,n] per head; V projection natural
     layout into vaug (bf16) with a ones-column per 65-wide head group so the
     P@V matmul also emits rowsum(P).
  2. pass1 per head: S[q,k] = qT.T @ kT (K=64); rowmax on DVE; PE-transpose of
     the [128,8] max matrix; DMA into row 64 of the augmented q tile.
  3. ST'[k,q] = kT_aug.T @ qT_aug with K=65 (row 64 = -1 / m[q] computes
     k.q - m[q] directly); exp on ACT (scale=8); OT_aug accumulated over kt
     with rowsum riding row 64.
  4. fc: out[n,d] = OT.T @ WfcT + b_fc.

Schedule (the performance core — PE must never idle or the HAM clock gate
re-throttles it from 2.4 GHz to 1.2 GHz):
  - Steady-state block h: [6 proj chunks h+2] [rs chain h-1] then kt 0..7 of
    {ST(h,kt); exp; one pass1 s_tile; PV(h,kt-2)}. PV runs TWO kt behind its
    exp so the PE never waits on ACT latency. psqk(h+2) evictions interleave
    on ACT/DVE at kts 0-2; mrow(h+1) fires at kt 4.
  - pass1(h) is split 5+3 across blocks h-2/h-1 so the m-row chain (last
    reduce -> transpose -> ACT copy -> sync-queue DMA) completes half a block
    before ST(h,0) consumes it.
  - psOT is released by a single 65-row DVE eviction (unnormalized OT + s);
    normalization is deferred one block: rs = exp(-ln s) computed on a
    [128,8] DRAM-bounce reshape (8-cycle ACT ops instead of 1024) for heads
    0-8, broadcast via zero-stride DRAM-source DMA, multiplied on the idle
    Pool engine. Heads 9-11 use a low-latency [1,1024] direct path on the
    sync DMA queue and DVE multiplies, because the fc consumes them
    immediately.
  - rs/s DMAs ride the GPSIMD SWDGE queue so they never head-of-line block
    the latency-critical m-row DMA on the sync queue.
  - fc: tile 0 prestarted inside head blocks 10/11 (chunks 0-3 banked to
    SBUF + bias, chunk 4 in block 11); tiles 1-7 pipeline chunk-5 one tile
    behind so nothing waits on the mul(11) chain. Redundant LDWEIGHTS of
    same-stationary matmul pairs are skipped (ins.ldweights=False).
"""

import os
import sys
from contextlib import ExitStack

import numpy as np

for _p in ("/opt/trn_rl_repo",):
    if _p not in sys.path and os.path.isdir(_p):
        sys.path.insert(0, _p)

import ml_dtypes  # noqa: E402

import concourse.bass as bass  # noqa: E402
import concourse.tile as tile  # noqa: E402
from concourse import mybir  # noqa: E402
from concourse.bass import ds, ts  # noqa: E402
from concourse.bass_utils import run_bass_kernel_spmd  # noqa: E402
from concourse.masks import make_identity  # noqa: E402

P = 128
NT = 1024   # tokens per core (batch element)
D = 768     # model dim
DC = D // P  # 6 contraction chunks
H = 12      # heads
DH = 64     # head dim
QT = NT // P  # 8 q tiles
KT = NT // P  # 8 k tiles
E3 = 3 * D  # 2304

F32 = mybir.dt.float32
F32R = mybir.dt.float32r
F16 = mybir.dt.float16
BF16 = mybir.dt.bfloat16

N_CORES = 8


def r(ap):
    """view an fp32 AP as float32r for full-speed PE matmul"""
    return ap.bitcast(F32R)


def no_ldw(inst):
    """Mark a matmul as reusing the stationary weights already loaded by the
    previous matmul (skips the redundant LDWEIGHTS; pairs sharing lhsT)."""
    inst.ins.ldweights = False
    return inst



def _split_sync_waits(nc, max_waits=1):
    """Walrus codegen allows only a limited number of semaphore waits per
    instruction (one for several instruction structs). Move extra waits onto
    same-engine NoOps inserted immediately before the offending instruction."""
    from concourse import mybir as mb
    for f in nc.m.functions:
        for b in f.blocks:
            out = []
            changed = False
            for inst in b.instructions:
                si = inst.sync_info
                waits = list(si.on_wait) if (si is not None and si.on_wait) else []
                eng = getattr(inst, "engine", None)
                if (type(inst).__name__ == "InstISA"
                        and getattr(inst, "op_name", None) == "EVENT_SEMAPHORE_RANGE_CLEAR"):
                    # walrus here rejects this opcode; emit per-sem resets instead
                    lo, hi = inst.instr[13], inst.instr[14]
                    for sid in range(lo, hi + 1):
                        out.append(mb.InstEventSemaphore(
                            name=nc.get_next_instruction_name(),
                            sync_info=mb.SyncInfo(on_wait=[], on_update=[
                                mb.SyncUpdate(sync_type="semaphore", id=sid,
                                              ant_name=f"semclr_{sid}",
                                              update_mode="sem-wr-imm",
                                              update_value=0, update_reg=None)]),
                            engine=eng,
                        ))
                    changed = True
                    continue
                lim = max_waits
                if len(waits) > lim and eng is not None:
                    for w in waits[:-lim]:
                        nop = mb.InstEventSemaphore(
                            name=nc.get_next_instruction_name(),
                            sync_info=mb.SyncInfo(on_wait=[w], on_update=[]),
                            engine=eng,
                        )
                        out.append(nop)
                    inst.sync_info = mb.SyncInfo(
                        on_wait=waits[-lim:],
                        on_update=list(si.on_update) if si.on_update else [],
                    )
                    changed = True
                out.append(inst)
            if changed:
                b.instructions = out


def build_graph():
    nc = bass.Bass()
    imgT = nc.declare_dram_parameter("imgT", [D, NT], F16, isOutput=False)
    WqkvT = nc.declare_dram_parameter("WqkvT", [D, E3], F16, isOutput=False)
    WfcT = nc.declare_dram_parameter("WfcT", [D, D], BF16, isOutput=False)
    b_fc = nc.declare_dram_parameter("b_fc", [D], F32, isOutput=False)
    out = nc.declare_dram_parameter("out", [NT, D], F32, isOutput=True)

    with tile.TileContext(nc) as tc, ExitStack() as ctx:
        const = ctx.enter_context(tc.tile_pool(name="const", bufs=1))
        ptp = ctx.enter_context(tc.tile_pool(name="ptp", bufs=4))
        small = ctx.enter_context(tc.tile_pool(name="small", bufs=2))
        rawp = ctx.enter_context(tc.tile_pool(name="rawp", bufs=3))
        rsp = ctx.enter_context(tc.tile_pool(name="rsp", bufs=3))
        outp = ctx.enter_context(tc.tile_pool(name="outp", bufs=3))
        # PSUM: two pools of 2 x [128,1024] slots = 8 banks total.
        # bigp: high-churn stream tiles (S, ST', mrow transpose), accp:
        # longer-lived accumulators (psqk proj, psOT).
        bigp = ctx.enter_context(tc.tile_pool(name="bigp", bufs=2, space="PSUM"))
        accp = ctx.enter_context(tc.tile_pool(name="accp", bufs=2, space="PSUM"))

        # ---- input loads: interleave img/wq chunks so the first projection
        # can start as soon as chunk 0 of each has landed; wf/bias last ----
        img_sb = []
        wq_sb = []
        wf_sb = []
        for c in range(DC):
            ti = const.tile([P, NT], F16, tag=f"img{c}", name=f"img{c}")
            nc.sync.dma_start(out=ti[:, :], in_=imgT[ds(c * P, P), :])
            img_sb.append(ti)
            tw = const.tile([P, E3], F16, tag=f"wq{c}", name=f"wq{c}")
            nc.sync.dma_start(out=tw[:, :], in_=WqkvT[ds(c * P, P), :])
            wq_sb.append(tw)
        for c in range(DC):
            t = const.tile([P, D], BF16, tag=f"wf{c}")
            nc.sync.dma_start(out=t[:, :], in_=WfcT[ds(c * P, P), :])
            wf_sb.append(t)

        bias_sb = const.tile([P, D], F32, tag="bias")
        b_ap = b_fc[:]
        b_bcast = bass.AP(tensor=b_ap.tensor, offset=b_ap.offset,
                          ap=[[0, P]] + list(b_ap.ap))
        nc.sync.dma_start(out=bias_sb[:, :], in_=b_bcast)

        # ---- HAM pre-warming ----
        # The PE clock-gate (HAM) releases 2.4 GHz only after ~3.4us of
        # sustained activity; engine init (~7.5us) + the input-DMA window
        # otherwise leave the PE cold+idle until ~12us, so the whole warmup
        # runs at 1.2 GHz. Burn the idle DMA-wait on dummy matmuls over a
        # zeroed tile (memset emitted first so nothing delays them): they
        # finish right as the first real chunk lands and the V projection
        # starts at full clock.
        warm_src = const.tile([P, 512], F16, tag="warm")
        nc.vector.memset(warm_src[:, :], 0.0)
        for i in range(16):
            psW = bigp.tile([P, 512], F32, tag="big", name=f"warm{i}")
            nc.tensor.matmul(psW[:, :], warm_src[:, 0:128], warm_src[:, :],
                             start=True, stop=True)

        ident = const.tile([P, P], F32, tag="ident")
        make_identity(nc, ident[:, :])

        # V with ones column per head: [k-part, kt, h*65 + c], col 64 of each
        # group = 1. Only the ones columns need initialization (the rest is
        # overwritten by the V-projection eviction) -> strided DVE memset.
        vaug = const.tile([P, KT, H * 65], BF16, tag="vaug")
        nc.vector.memset(
            vaug[:, :, :].rearrange("p t (h c) -> p t h c", c=65)[:, :, :, 64:65],
            1.0)

        # merged attention output, transposed, one tile per fc contraction
        # chunk (separate tiles so fc chunk c only depends on the two head
        # muls that write it, not on every head via coarse tile tracking)
        otc = [const.tile([P, NT], BF16, tag=f"otc{c}", name=f"otc{c}")
               for c in range(DC)]

        # persistent triple-buffered q/k tiles (parity h%3; three-deep so the
        # mid-block eviction of head h+2 never overlaps head h's readers).
        # ka row 64 = -1 (set once); qa row 64 = rowmax, DMA'd in by mrow().
        qa_bufs = [const.tile([65, NT], F16, tag=f"qab{i}", name=f"qab{i}")
                   for i in range(3)]
        ka_bufs = [const.tile([65, NT], F16, tag=f"kab{i}", name=f"kab{i}")
                   for i in range(3)]
        for i in range(3):
            nc.vector.memset(ka_bufs[i][64:65, :], -1.0)

        # ---- V projection (natural layout) ----
        for t in range(QT):
            psv = bigp.tile([P, D], F32, tag="big", name=f"psv{t}")
            for c in range(DC):
                lt = img_sb[c][:, ts(t, P)]
                wv = wq_sb[c][:, :].rearrange("p (h x) -> p h x", h=H)
                nc.tensor.matmul(psv[:, 0:512].rearrange("p (h x) -> p h x", h=8),
                                 lt, wv[:, 0:8, 128:192],
                                 start=(c == 0), stop=(c == DC - 1))
                no_ldw(nc.tensor.matmul(
                    psv[:, 512:768].rearrange("p (h x) -> p h x", h=4),
                    lt, wv[:, 8:12, 128:192],
                    start=(c == 0), stop=(c == DC - 1)))
            nc.scalar.copy(
                vaug[:, t, :].rearrange("p (h x) -> p h x", h=H)[:, :, 0:64],
                psv[:, :].rearrange("p (h x) -> p h x", h=H))

        def proj_alloc(h):
            return accp.tile([P, NT], F32, tag="acc", name=f"psqk{h}")

        def proj_chunk(psqk, h, c):
            lt = wq_sb[c][:, ds(h * 192, P)]
            for nb in range(2):
                i = nc.tensor.matmul(psqk[:, ts(nb, 512)], lt,
                                     img_sb[c][:, ts(nb, 512)],
                                     start=(c == 0), stop=(c == DC - 1))
                if nb:
                    no_ldw(i)

        def proj_evict(psqk, h):
            """fp32 PSUM -> fp16 qa/ka; split across ACT and DVE."""
            qa = qa_bufs[h % 3]
            ka = ka_bufs[h % 3]
            nc.scalar.copy(qa[0:64, :], psqk[0:64, :])
            nc.vector.tensor_copy(ka[0:64, :], psqk[64:128, :])
            return qa, ka

        def evict_qa_half(psqk, h, half):
            """half-width ACT eviction, interleaved between exp ops so the
            softmax stream never stalls more than one exp slot."""
            qa = qa_bufs[h % 3]
            nc.scalar.copy(qa[0:64, ts(half, 512)], psqk[0:64, ts(half, 512)])

        def evict_ka(psqk, h):
            ka = ka_bufs[h % 3]
            nc.vector.tensor_copy(ka[0:64, :], psqk[64:128, :])
            return qa_bufs[h % 3], ka

        def proj_head(h):
            psqk = proj_alloc(h)
            for c in range(DC):
                proj_chunk(psqk, h, c)
            return proj_evict(psqk, h)

        def s_tile(t, qa, ka, mcat, nm=""):
            """S[q,k] for one q-tile; rowmax on DVE. (tensor_tensor_reduce on
            the two halves would halve the cycles but the ISA allows only one
            PSUM input per DVE instruction.)"""
            psS = bigp.tile([P, NT], F32, tag="big", name=f"psS{nm}_{t}")
            lt = qa[0:64, ts(t, P)]
            for nb in range(2):
                i = nc.tensor.matmul(psS[:, ts(nb, 512)], lt,
                                     ka[0:64, ts(nb, 512)],
                                     start=True, stop=True)
                if nb:
                    no_ldw(i)
            nc.vector.reduce_max(out=mcat[:, ds(t, 1)], in_=psS[:, :],
                                 axis=mybir.AxisListType.X)

        def mrow(h, qa, mcat):
            """m [128,8] -> transposed row [1,1024] into qa row 64"""
            psT = bigp.tile([8, P], F32, tag="big", name=f"psT{h}")
            nc.tensor.transpose(psT[:, :], mcat[:, :], ident[:, :])
            m_sb = small.tile([8, P], F16, tag="mrow")
            nc.scalar.copy(m_sb[:, :], psT[:, :])
            nc.sync.dma_start(
                out=qa[ds(64, 1), :].rearrange("o (t x) -> o t x", t=QT),
                in_=m_sb[:, :])

        raws = {}
        rs64s = {}

        def rs_chain(h, psOT, dma_eng=None):
            """Free psOT fast with one 65-row DVE eviction (OT + rowsum s).
            rs = 1/s via exp(-ln s), but computed on a [128, 8] RESHAPE of the
            s row (DMA bounce through DRAM) so the two ACT ops cost 8 cycles
            of free dim instead of 1024 -- the ACT engine stays exp-bound.
            Then flatten + partition-broadcast back (DRAM source allows the
            zero-stride partition dim). All DMAs ride the GPSIMD SWDGE queue
            (sync queue for the last head, which feeds the fc with the sync
            queue idle) so they never block the m-row DMA."""
            raw = rawp.tile([65, NT], BF16, tag="raw", name=f"raw{h}")
            nc.vector.tensor_copy(raw[:, 0:512], psOT[:, 0:512])
            nc.scalar.copy(raw[:, 512:1024], psOT[:, 512:1024])
            raws[h] = raw
            eng = dma_eng if dma_eng is not None else nc.gpsimd
            if h >= 9:
                # low-latency direct path: Ln straight off the PSUM row (no
                # raw dependency), one bounce + one broadcast. These heads
                # gate the fc prestart/tail.
                lns1 = small.tile([1, NT], F32, tag="lns1")
                nc.scalar.activation(lns1[:, :], psOT[ds(64, 1), :],
                                     mybir.ActivationFunctionType.Ln,
                                     bias=0.0, scale=1.0)
                rs1 = small.tile([1, NT], F16, tag="rs1")
                nc.scalar.activation(rs1[:, :], lns1[:, :],
                                     mybir.ActivationFunctionType.Exp,
                                     bias=0.0, scale=-1.0)
                rd = nc.dram_tensor(f"rs_dram{h}", [NT], F16, kind="Internal")
                eng.dma_start(out=rd[:], in_=rs1[:, :])
                rs64d = rsp.tile([DH, NT], F16, tag="rs64", name=f"rs64_{h}")
                dd = rd[:]
                bcd = bass.AP(tensor=dd.tensor, offset=dd.offset,
                              ap=[[0, DH]] + list(dd.ap))
                eng.dma_start(out=rs64d[:, :], in_=bcd)
                rs64s[h] = rs64d
                return
            s_dram = nc.dram_tensor(f"s_dram{h}", [NT], BF16, kind="Internal")
            eng.dma_start(out=s_dram[:], in_=raw[ds(64, 1), :])
            s128 = small.tile([P, QT], BF16, tag="s128")
            eng.dma_start(out=s128[:, :], in_=s_dram[:])
            lns = small.tile([P, QT], F32, tag="lns")
            nc.scalar.activation(lns[:, :], s128[:, :],
                                 mybir.ActivationFunctionType.Ln,
                                 bias=0.0, scale=1.0)
            rs128 = small.tile([P, QT], F16, tag="rs128")
            nc.scalar.activation(rs128[:, :], lns[:, :],
                                 mybir.ActivationFunctionType.Exp,
                                 bias=0.0, scale=-1.0)
            rs_dram = nc.dram_tensor(f"rs_dram{h}", [NT], F16, kind="Internal")
            eng.dma_start(out=rs_dram[:], in_=rs128[:, :])
            rs64 = rsp.tile([DH, NT], F16, tag="rs64", name=f"rs64_{h}")
            d_ap = rs_dram[:]
            r_bcast = bass.AP(tensor=d_ap.tensor, offset=d_ap.offset,
                              ap=[[0, DH]] + list(d_ap.ap))
            eng.dma_start(out=rs64[:, :], in_=r_bcast)
            rs64s[h] = rs64

        def mul_head(h):
            """deferred normalization multiply: heads 0-8 on the idle Pool
            engine (one-block latency budget), 9-11 on DVE (the fc prestart
            and tail consume them with little slack)."""
            dst = otc[h // 2][ds((h % 2) * 64, DH), :]
            if h <= 8:
                nc.gpsimd.tensor_mul(dst, raws[h][0:64, :], rs64s[h][:, :])
            else:
                nc.vector.tensor_mul(dst, raws[h][0:64, :], rs64s[h][:, :])
            del raws[h], rs64s[h]

        def pv_mms(h, kt, pt_t, psOT):
            for nb in range(2):
                i = nc.tensor.matmul(psOT[:, ts(nb, 512)],
                                     vaug[:, kt, ds(h * 65, 65)],
                                     pt_t[:, ts(nb, 512)],
                                     start=(kt == 0), stop=(kt == KT - 1))
                if nb:
                    no_ldw(i)

        # ---- warmup: heads 0 and 1 projected; pass1(0) complete (+mrow),
        # pass1(1) tiles 0-4 done (remainder lands in block 0) ----
        qa_ka = {}
        mcats = {}
        s_q = {}
        qa_ka[0] = proj_head(0)
        mcats[0] = small.tile([P, QT], F32, tag="mcat", name="mcat0")
        for t in range(QT):
            s_tile(t, qa_ka[0][0], qa_ka[0][1], mcats[0], nm="w")
        mrow(0, qa_ka[0][0], mcats[0])
        qa_ka[1] = proj_head(1)
        mcats[1] = small.tile([P, QT], F32, tag="mcat", name="mcat1")
        s_q[1] = list(range(QT))
        for _ in range(5):
            t = s_q[1].pop(0)
            s_tile(t, qa_ka[1][0], qa_ka[1][1], mcats[1], nm="1")

        # fc-prestart state (tile 0 of the fc is computed early to give the
        # last two head blocks PE filler): fcpart0 = chunks 0-3 + bias.
        fcpart0 = const.tile([P, D], F32, tag="fcpart0")
        fc_state = {}
        pending_rs = {}

        def fc_chunk(psF, t, c, start, stop):
            nc.tensor.matmul(psF[:, 0:512], otc[c][:, ts(t, P)],
                             wf_sb[c][:, 0:512], start=start, stop=stop)
            no_ldw(nc.tensor.matmul(psF[:, 512:768], otc[c][:, ts(t, P)],
                                    wf_sb[c][:, 512:768],
                                    start=start, stop=stop))

        # ---- steady-state head blocks ----
        # Block h (PE program order):
        #   pre: [6 proj chunks h+2] [rs chain h-1 (ACT/DVE/DMA only)]
        #   kt 0..2: ST | s_tile(h+1) remainder | PV        (+ psqk(h+2)
        #     evictions on ACT/DVE at kts 1-2, ka first)
        #   kt 3..7: ST | s_tile(h+2) tiles 0..4 | PV; mrow(h+1) at kt 4.
        #   tail: deferred norm muls.
        # pass1(h) is split 5+3 across blocks h-2/h-1 so the m-row chain
        # (last reduce -> PE transpose -> ACT copy -> DMA) completes half a
        # block before ST(h,0) consumes qa(h) row 64.
        for h in range(H):
            qa, ka = qa_ka[h]
            nxt = qa_ka.get(h + 1)
            prj = h + 2 if h + 2 < H else None
            # accp allocation order per block must stay strictly alternating:
            # normal blocks [psqk(h+2), psOT(h)]; h=10 [psOT, psF0];
            # h=11 [psOT, psF0b].
            psqk2 = proj_alloc(prj) if prj is not None else None
            if psqk2 is None:
                psOT = accp.tile([65, NT], F32, tag="acc", name=f"psOT{h}")
                psF0 = accp.tile([P, D], F32, tag="acc", name=f"psF0_{h}")
                fc_state[h] = psF0
            # --- pre-region ---
            if psqk2 is not None:
                for c in range(DC):
                    proj_chunk(psqk2, prj, c)
            elif h == 10:
                psF0 = fc_state[10]
                for c in range(4):
                    fc_chunk(psF0, 0, c, start=(c == 0), stop=(c == 3))
            elif h == 11:
                psF0b = fc_state[11]
                fc_chunk(psF0b, 0, 4, start=True, stop=False)
            if (h - 1) in pending_rs:
                rs_chain(h - 1, pending_rs.pop(h - 1))
            if psqk2 is not None:
                psOT = accp.tile([65, NT], F32, tag="acc", name=f"psOT{h}")
            if prj is not None:
                mcats[prj] = small.tile([P, QT], F32, tag="mcat",
                                        name=f"mcat{prj}")
                s_q[prj] = list(range(QT))
            # --- softmax/PV pipeline ---
            pts = {}
            for kt in range(KT):
                psST = bigp.tile([P, NT], F32, tag="big", name=f"psST{h}_{kt}")
                lt = ka[:, ts(kt, P)]
                for nb in range(2):
                    i = nc.tensor.matmul(psST[:, ts(nb, 512)], lt,
                                         qa[:, ts(nb, 512)],
                                         start=True, stop=True)
                    if nb:
                        no_ldw(i)
                pt_t = ptp.tile([P, NT], BF16, tag="pt")
                nc.scalar.activation(pt_t[:, :], psST[:, :],
                                     mybir.ActivationFunctionType.Exp,
                                     bias=0.0, scale=8.0)
                pts[kt] = pt_t
                # fillers: kts 0-2 finish pass1(h+1); kts 3-7 run the first
                # five tiles of pass1(h+2)
                if kt < 3:
                    if nxt is not None and s_q.get(h + 1):
                        t = s_q[h + 1].pop(0)
                        s_tile(t, nxt[0], nxt[1], mcats[h + 1], nm=str(h + 1))
                elif prj is not None and s_q.get(prj):
                    t = s_q[prj].pop(0)
                    s_tile(t, qa_ka[prj][0], qa_ka[prj][1], mcats[prj],
                           nm=str(prj))
                elif nxt is None and kt < KT - 1:
                    psW = bigp.tile([P, 512], F32, tag="big",
                                    name=f"warm11_{kt}")
                    nc.tensor.matmul(psW[:, :], warm_src[:, 0:128],
                                     warm_src[:, :], start=True, stop=True)
                if psqk2 is not None:
                    if kt == 0:
                        qa_ka[prj] = evict_ka(psqk2, prj)
                    elif kt == 1:
                        evict_qa_half(psqk2, prj, 0)
                    elif kt == 2:
                        evict_qa_half(psqk2, prj, 1)
                if kt == 4 and nxt is not None:
                    mrow(h + 1, nxt[0], mcats[h + 1])
                if kt > 1:
                    pv_mms(h, kt - 2, pts[kt - 2], psOT)
                    del pts[kt - 2]
            pv_mms(h, KT - 2, pts[KT - 2], psOT)
            pv_mms(h, KT - 1, pts[KT - 1], psOT)
            # --- tail ---
            if h == H - 1:
                rs_chain(h, psOT, dma_eng=nc.sync)
            else:
                pending_rs[h] = psOT
            if h == 10:
                # bank partial fc tile 0 (chunks 0-3 + bias) into SBUF so the
                # accp slot frees this block (avoids a psOT(11) ring stall)
                nc.vector.tensor_add(fcpart0[:, :], fc_state[10][:, :],
                                     bias_sb[:, :])
            if h > 0:
                mul_head(h - 1)

        # ---- fc + bias, two-phase ----
        # Phase 1: chunks 0-3 (+bias) for tiles 1-7, banked to SBUF — no
        # dependency on the late heads, so ~12us of PE work hides the
        # mul(11) normalization chain completely. mul(11) is emitted AFTER
        # the phase-1 adds so it never head-of-line blocks the DVE queue.
        # Phase 2: chunks 4-5 + combine with the banked partial. Tile 0's
        # phase 1 ran as filler inside head blocks 10/11.
        fcparts = {0: fcpart0}

        def fc_phase1(t):
            psF = bigp.tile([P, D], F32, tag="big", name=f"psF{t}")
            for c in range(4):
                fc_chunk(psF, t, c, start=(c == 0), stop=(c == 3))
            fp = const.tile([P, D], F32, tag=f"fcp{t}", name=f"fcp{t}")
            nc.vector.tensor_add(fp[:, :], psF[:, :], bias_sb[:, :])
            fcparts[t] = fp

        def fc_phase2(t):
            if t == 0:
                psF = fc_state[11]  # chunk 4 accumulated in block 11's pre
            else:
                psF = bigp.tile([P, D], F32, tag="big", name=f"psF2_{t}")
                fc_chunk(psF, t, 4, start=True, stop=False)
            fc_chunk(psF, t, 5, start=False, stop=True)
            o_t = outp.tile([P, D], F32, tag="o", name=f"o_t{t}")
            nc.vector.tensor_add(o_t[:, :], psF[:, :], fcparts[t][:, :])
            nc.sync.dma_start(out=out[ts(t, P), :], in_=o_t[:, :])

        for t in range(1, QT):
            fc_phase1(t)
            if t == 4:
                # mid-phase emission: late enough that the rs chain is ready
                # (no DVE head-of-line block), early enough to finish before
                # the PE reaches phase 2
                mul_head(H - 1)
        for t in range(QT):
            fc_phase2(t)

    _split_sync_waits(nc)
    return nc


_NC_CACHE = {}


def _get_graph():
    if "nc" not in _NC_CACHE:
        _NC_CACHE["nc"] = build_graph()
    return _NC_CACHE["nc"]


_EXEC_CACHE = {}


def _install_compile_memo():
    import hashlib
    import shutil
    from concourse import bass_utils as bu
    from concourse import bass2jax
    if getattr(bu.compile_bir_kernel, "_memo", False):
        return
    orig = bu.compile_bir_kernel

    def memo_compile(bir_json, tmpdir, neff_name="file.neff"):
        key = hashlib.sha256(bir_json).hexdigest()
        os.makedirs("/tmp/neff_cache", exist_ok=True)
        persist = f"/tmp/neff_cache/{key}.neff"
        if os.path.exists(persist):
            return persist
        r = orig(bir_json, tmpdir, neff_name)
        shutil.copyfile(r, persist)
        return persist
    memo_compile._memo = True
    bu.compile_bir_kernel = memo_compile
    bass2jax.compile_bir_kernel = memo_compile


def _get_executor():
    _install_compile_memo()
    """Build (once) a jitted shard_map executor over 8 cores, non-donating so
    it can be re-invoked for benchmarking. Mirrors bass2jax.run_bass_via_pjrt."""
    if "exec" in _EXEC_CACHE:
        return _EXEC_CACHE["exec"]
    import jax
    import jax.numpy as jnp
    from jax.sharding import Mesh, PartitionSpec
    from jax.experimental.shard_map import shard_map
    from concourse import mybir as mb
    from concourse import bass2jax

    bass2jax.install_neuronx_cc_hook()
    nc = _get_graph()
    partition_name = (nc.partition_id_tensor.name
                      if nc.partition_id_tensor else None)
    in_names, out_names, out_avals, zero_outs = [], [], [], []
    for alloc in nc.m.functions[0].allocations:
        if not isinstance(alloc, mb.MemoryLocationSet):
            continue
        name = alloc.memorylocations[0].name
        if alloc.kind == "ExternalInput":
            if name != partition_name:
                in_names.append(name)
        elif alloc.kind == "ExternalOutput":
            shape = tuple(alloc.tensor_shape)
            dtype = mb.dt.np(alloc.dtype)
            out_names.append(name)
            out_avals.append(jax.core.ShapedArray(shape, dtype))
            zero_outs.append(np.zeros(shape, dtype))
    n_params = len(in_names)
    all_in_names = list(in_names) + list(out_names)
    if partition_name is not None:
        all_in_names.append(partition_name)

    def _body(*args):
        operands = list(args)
        if partition_name is not None:
            operands.append(bass2jax.partition_id_tensor())
        outs = bass2jax._bass_exec_p.bind(
            *operands,
            out_avals=tuple(out_avals),
            in_names=tuple(all_in_names),
            out_names=tuple(out_names),
            lowering_input_output_aliases=(),
            sim_require_finite=True,
            sim_require_nnan=True,
            nc=nc,
        )
        return tuple(outs)

    devices = jax.devices()[:N_CORES]
    mesh = Mesh(np.asarray(devices), ("core",))
    n_outs = len(out_names)
    in_specs = (PartitionSpec("core"),) * (n_params + n_outs)
    out_specs = (PartitionSpec("core"),) * n_outs
    sharded = jax.jit(shard_map(_body, mesh=mesh, in_specs=in_specs,
                                out_specs=out_specs, check_rep=False))
    ex = dict(fn=sharded, in_names=in_names, out_names=out_names,
              out_avals=out_avals, zero_outs=zero_outs, n_params=n_params)
    _EXEC_CACHE["exec"] = ex
    return ex


def _prep_inputs(img, W_qkv, W_fc, b_fc):
    img = np.asarray(img, dtype=np.float32)
    W_qkv = np.asarray(W_qkv, dtype=np.float32)
    W_fc = np.asarray(W_fc, dtype=np.float32)
    b_fc = np.asarray(b_fc, dtype=np.float32)
    imgT = np.ascontiguousarray(img.transpose(0, 2, 1)).astype(np.float16)
    WqkvT = np.ascontiguousarray(W_qkv.T).astype(np.float16)
    WfcT = np.ascontiguousarray(W_fc.T).astype(ml_dtypes.bfloat16)
    return [{"imgT": imgT[i], "WqkvT": WqkvT, "WfcT": WfcT, "b_fc": b_fc}
            for i in range(N_CORES)]


def _run_cached(in_maps):
    ex = _get_executor()
    n_cores = N_CORES
    per_core = [[np.asarray(m[name]) for name in ex["in_names"]]
                for m in in_maps]
    concat_in = [np.concatenate([per_core[c][i] for c in range(n_cores)], axis=0)
                 for i in range(ex["n_params"])]
    concat_zeros = [np.zeros((n_cores * z.shape[0], *z.shape[1:]), z.dtype)
                    for z in ex["zero_outs"]]
    out_arrs = ex["fn"](*concat_in, *concat_zeros)
    outs = [
        {name: np.asarray(out_arrs[i]).reshape(n_cores, *ex["out_avals"][i].shape)[c]
         for i, name in enumerate(ex["out_names"])}
        for c in range(n_cores)
    ]
    return outs


def bench(n_iters=20):
    """Wall-clock benchmark of the compiled executable (inputs device-resident
    once; n_iters sequential executes, block at the end)."""
    import time
    import jax
    inputs_np = None
    try:
        z = np.load("/root/problem/_expected.npz")
        inputs_np = {k: z[k] for k in ("img", "W_qkv", "W_fc", "b_fc")}
    except Exception:
        rng = np.random.default_rng(0)
        inputs_np = {
            "img": rng.standard_normal((8, 1024, 768), dtype=np.float32),
            "W_qkv": rng.standard_normal((E3, D), dtype=np.float32),
            "W_fc": rng.standard_normal((D, D), dtype=np.float32),
            "b_fc": rng.standard_normal((D,), dtype=np.float32),
        }
    in_maps = _prep_inputs(**inputs_np)
    ex = _get_executor()
    per_core = [[np.asarray(m[name]) for name in ex["in_names"]] for m in in_maps]
    concat_in = [np.concatenate([per_core[c][i] for c in range(N_CORES)], axis=0)
                 for i in range(ex["n_params"])]
    concat_zeros = [np.zeros((N_CORES * z.shape[0], *z.shape[1:]), z.dtype)
                    for z in ex["zero_outs"]]
    # warmup + compile
    o = ex["fn"](*concat_in, *concat_zeros)
    jax.block_until_ready(o)
    # sequential, block each call
    ts = []
    for _ in range(n_iters):
        t0 = time.perf_counter()
        o = ex["fn"](*concat_in, *concat_zeros)
        jax.block_until_ready(o)
        ts.append(time.perf_counter() - t0)
    # pipelined: fire all, block once
    t0 = time.perf_counter()
    os_ = [ex["fn"](*concat_in, *concat_zeros) for _ in range(n_iters)]
    jax.block_until_ready(os_)
    piped = (time.perf_counter() - t0) / n_iters
    return dict(min_s=min(ts), mean_s=sum(ts) / len(ts), piped_s=piped)


def _run(img, W_qkv, W_fc, b_fc, trace=False, tmpdir=None):
    in_maps = _prep_inputs(img, W_qkv, W_fc, b_fc)
    results = _run_cached(in_maps)
    outs = np.stack([np.asarray(results[i]["out"], dtype=np.float32)
                     for i in range(N_CORES)])
    return outs, None


def kernel(img, W_qkv, W_fc, b_fc):
    outs, _ = _run(img, W_qkv, W_fc, b_fc)
    return outs


def bench_chain(n=9, reps=5):
    """Real-HW per-iteration time: jit a chain of n dependent kernel
    executions (out buffer of call i feeds call i+1). Slope = (t_n - t_1)/(n-1).
    Walrus compiles are memoized by BIR hash so the chain compiles once."""
    import time
    import hashlib
    import jax
    from jax.sharding import Mesh, PartitionSpec
    from jax.experimental.shard_map import shard_map
    from concourse import mybir as mb
    from concourse import bass2jax
    from concourse import bass_utils as bu

    if not hasattr(bu.compile_bir_kernel, "_memo"):
        orig = bu.compile_bir_kernel

        def memo_compile(bir_json, tmpdir, neff_name="file.neff"):
            import shutil
            key = hashlib.sha256(bir_json).hexdigest()
            cache = memo_compile._cache
            if key in cache:
                return cache[key]
            r = orig(bir_json, tmpdir, neff_name)
            os.makedirs("/tmp/neff_cache", exist_ok=True)
            persist = f"/tmp/neff_cache/{key}.neff"
            shutil.copyfile(r, persist)
            cache[key] = persist
            return persist
        memo_compile._cache = {}
        memo_compile._memo = True
        bu.compile_bir_kernel = memo_compile
        bass2jax.compile_bir_kernel = memo_compile

    bass2jax.install_neuronx_cc_hook()
    nc = _get_graph()
    partition_name = (nc.partition_id_tensor.name
                      if nc.partition_id_tensor else None)
    in_names, out_names, out_avals = [], [], []
    for alloc in nc.m.functions[0].allocations:
        if not isinstance(alloc, mb.MemoryLocationSet):
            continue
        name = alloc.memorylocations[0].name
        if alloc.kind == "ExternalInput":
            if name != partition_name:
                in_names.append(name)
        elif alloc.kind == "ExternalOutput":
            out_names.append(name)
            out_avals.append(jax.core.ShapedArray(
                tuple(alloc.tensor_shape), mb.dt.np(alloc.dtype)))
    n_params = len(in_names)
    all_in = list(in_names) + list(out_names)
    if partition_name is not None:
        all_in.append(partition_name)

    def make_body(n_iter):
        def _body(*args):
            ins = list(args[:n_params])
            outb = list(args[n_params:])
            outs = None
            for _ in range(n_iter):
                operands = ins + outb
                if partition_name is not None:
                    operands.append(bass2jax.partition_id_tensor())
                outs = bass2jax._bass_exec_p.bind(
                    *operands,
                    out_avals=tuple(out_avals),
                    in_names=tuple(all_in),
                    out_names=tuple(out_names),
                    lowering_input_output_aliases=(),
                    sim_require_finite=False,
                    sim_require_nnan=False,
                    nc=nc,
                )
            return tuple(outs)
        return _body

    z = np.load("/root/problem/_expected.npz")
    in_maps = _prep_inputs(z["img"], z["W_qkv"], z["W_fc"], z["b_fc"])
    per_core = [[np.asarray(m[k]) for k in in_names] for m in in_maps]
    concat_in = [np.concatenate([per_core[c][i] for c in range(N_CORES)], axis=0)
                 for i in range(n_params)]
    concat_zeros = [np.zeros((N_CORES * a.shape[0], *a.shape[1:]), a.dtype)
                    for a in out_avals]
    devices = jax.devices()[:N_CORES]
    mesh = Mesh(np.asarray(devices), ("core",))
    res = {}
    for n_iter in (1, n):
        body = make_body(n_iter)
        fn = jax.jit(shard_map(body, mesh=mesh,
                               in_specs=(PartitionSpec("core"),) * (n_params + len(out_names)),
                               out_specs=(PartitionSpec("core"),) * len(out_names),
                               check_rep=False))
        o = fn(*concat_in, *concat_zeros)
        jax.block_until_ready(o)  # warm
        ts = []
        for _ in range(reps):
            t0 = time.perf_counter()
            o = fn(*concat_in, *concat_zeros)
            jax.block_until_ready(o)
            ts.append(time.perf_counter() - t0)
        res[n_iter] = min(ts)
        print(f"chain n={n_iter}: min {min(ts)*1e3:.2f} ms over {reps} reps")
    per_iter = (res[n] - res[1]) / (n - 1)
    print(f"per-iteration (HW exec) ~= {per_iter*1e6:.1f} us")
    return per_iter


def bench_resident(m1=10, m2=40):
    """Per-call cost with device-resident inputs and a single executable:
    slope between m1 and m2 sequential async dispatches."""
    import time
    import jax
    from jax.sharding import Mesh, PartitionSpec, NamedSharding
    ex = _get_executor()
    z = np.load("/root/problem/_expected.npz")
    in_maps = _prep_inputs(z["img"], z["W_qkv"], z["W_fc"], z["b_fc"])
    per_core = [[np.asarray(m[k]) for k in ex["in_names"]] for m in in_maps]
    concat_in = [np.concatenate([per_core[c][i] for c in range(N_CORES)], axis=0)
                 for i in range(ex["n_params"])]
    concat_zeros = [np.zeros((N_CORES * z_.shape[0], *z_.shape[1:]), z_.dtype)
                    for z_ in ex["zero_outs"]]
    devices = jax.devices()[:N_CORES]
    mesh = Mesh(np.asarray(devices), ("core",))
    sh = NamedSharding(mesh, PartitionSpec("core"))
    dev_in = [jax.device_put(a, sh) for a in concat_in]
    dev_zero = [jax.device_put(a, sh) for a in concat_zeros]
    jax.block_until_ready(dev_in + dev_zero)
    fn = ex["fn"]
    o = fn(*dev_in, *dev_zero)
    jax.block_until_ready(o)
    res = {}
    for m in (m1, m2):
        best = None
        for _ in range(3):
            t0 = time.perf_counter()
            outs = [fn(*dev_in, *dev_zero) for _ in range(m)]
            jax.block_until_ready(outs)
            dt = time.perf_counter() - t0
            best = dt if best is None else min(best, dt)
        res[m] = best
        print(f"m={m}: {best*1e3:.2f} ms total, {best/m*1e3:.3f} ms/call")
    slope = (res[m2] - res[m1]) / (m2 - m1)
    print(f"slope (per-call device cost) ~= {slope*1e6:.1f} us")
    return slope

